# revision 41
# baseline (speedup 1.0000x reference)
"""Trainium2 Bass kernel for EnhancedLinkPredictor (GNN common-neighbor link prediction).

Math (per prediction edge e=(s,d)):
  shared_ddi = adj_ddi[s] & adj_ddi[d]          (drug-drug, N_D=8192)
  cn_ddi     = (shared_ddi @ z_drug)  / max(|shared_ddi|, 1)
  shared_dp  = adj_dp[s]  & adj_dp[d]           (drug-protein, N_P=4096)
  cn_prot    = (shared_dp @ z_protein) / max(|shared_dp|, 1)
  pair  = [z_drug[s], z_drug[d], cn_ddi, cn_prot]   (256)
  out   = sigmoid(relu(pair @ W1 + b1) @ W2 + b2)

Device strategy (8 cores, data-parallel over the 16384 pred edges, 2048/core):
  - No on-device gathers. pred_ei is host-known, so the host materializes the
    s-side and d-side adjacency rows in EDGE ORDER, 1-bit packed and
    pre-transposed to the SBUF layout: partition p's u16 word for superchunk C
    holds neighbors j = 2048*C + 16*p + 8*i + b at bit (8*i + b). Rows are
    streamed with plain HWDGE dma_start on one priority-ordered queue.
  - Device AND: one tensor_tensor(bitwise_and) per supertile computes s&d
    over the packed bits ([128, 6 words, E] u16).
  - Bit->fp8 expansion: for each bit-pair b (8 planes/supertile), ONE fused
    tensor_scalar((and <<|>> sh_b) & 0x0808) produces the fp8 e4m3 plane
    directly (byte 0x08 = 2^-6); bits (b, 8+b) land at byte positions (0, 1)
    = the DoubleRow 'two' sub-rows. Runs in the DVE 4x perf mode.
  - cn matmuls: fp8 DoubleRow, stationary [128, 2, 128] holding
    [count | hi limb of 4*z (64) | lo limb dims 0..62]; hi+lo summation
    happens inside the MLP W1 matmul via duplicated W1 rows. Stationaries
    are laid out b-major and DMA'd in per-b chunks so the first matmuls'
    weights arrive first.
  - Normalize: counts sit in PSUM row 0; DVE max+reciprocal on [1, 2E],
    gpsimd broadcasts 1/cnt to 128 partitions and also does the psum*rec
    multiplies (keeping the DVE free for mask planes).
  - Supertile sizes (256, 512, 512, 512, 256): small first tile starts the
    PE early; small last tile shortens the serial epilogue tail.
"""

import numpy as np
import ml_dtypes
from contextlib import ExitStack

import concourse.bass as bass
import concourse.bacc as bacc
import concourse.mybir as mybir
import concourse.tile as tile

N_D, N_P = 8192, 4096
D_DIM, HID = 64, 128
E_PRED = 16384
N_CORES = 8
E_LOC = E_PRED // N_CORES          # 2048 edges per core
STS = (512, 512, 512, 512)         # supertile sizes
OFFS = tuple(int(x) for x in np.cumsum((0,) + STS))  # edge offsets
N_ET = len(STS)
C_A = N_D // 2048                  # 4 ddi superchunks (2048 entries each)
C_P = N_P // 2048                  # 2 dp superchunks
W_TOT = C_A + C_P                  # 6 u16 words per partition per row
G_A = C_A * 8                      # 32 ddi stationary groups
G_P = C_P * 8                      # 16 dp stationary groups

S_OUT = 2.0 ** -4                  # plane code (2^-6) * z premult (4) product
Z_SCALE = 4.0

FP8 = ml_dtypes.float8_e4m3
BF16 = ml_dtypes.bfloat16


def _pack_stationary(z: np.ndarray):
    """z [K, 64] f32 -> [128, (K/2048)*8*2*128] uint8 fp8 DoubleRow lhsT.
    B-MAJOR group order: g = b*(K/2048) + C holds rows k = 2048C + 16p + 8i + b
    at (partition p, sub-row i), scaled by Z_SCALE. Cols: 0 = count (Z_SCALE),
    [1:65] hi limb, [65:128] lo limb of dims 0..62."""
    K = z.shape[0]
    nC = K // 2048
    p = np.arange(128)[:, None]
    i = np.arange(2)[None, :]
    out = np.empty((8 * nC, 128, 2, 128), dtype=np.uint8)
    code = np.float32(Z_SCALE).astype(FP8).view(np.uint8)
    for C in range(nC):
        for b in range(8):
            ks = 2048 * C + 16 * p + 8 * i + b            # [128, 2]
            zsc = z[ks].astype(np.float32) * Z_SCALE      # [128, 2, 64]
            hi8 = zsc.astype(FP8)
            lo8 = (zsc - hi8.astype(np.float32)).astype(FP8)
            blk = np.zeros((128, 2, 128), dtype=np.uint8)
            blk[..., 0] = code
            blk[..., 1:65] = hi8.view(np.uint8)
            blk[..., 65:128] = lo8.view(np.uint8)[..., :63]
            out[b * nC + C] = blk
    return np.ascontiguousarray(out.transpose(1, 0, 2, 3).reshape(128, -1))


def build_body(tc, t):
    """Emit the per-core program. t: dict name -> AP of DRAM tensors."""
    nc = tc.nc
    dt = mybir.dt
    with ExitStack() as ctx:
        const = ctx.enter_context(tc.tile_pool(name="const", bufs=1))
        rows = ctx.enter_context(tc.tile_pool(name="rows", bufs=3))
        andp = ctx.enter_context(tc.tile_pool(name="andp", bufs=2))
        plnp = ctx.enter_context(tc.tile_pool(name="plnp", bufs=10))
        tails = ctx.enter_context(tc.tile_pool(name="tails", bufs=2))
        ps_cn = ctx.enter_context(tc.tile_pool(name="ps_cn", bufs=2, space="PSUM"))
        ps_ml = ctx.enter_context(tc.tile_pool(name="ps_ml", bufs=2, space="PSUM"))

        # Everything on the single sync HWDGE queue, in need-order.
        zds = const.tile([128, G_A * 2 * 128], dt.uint8)
        zps = const.tile([128, G_P * 2 * 128], dt.uint8)
        w1t = const.tile([128, 4 * HID], dt.uint16)
        w2t = const.tile([128, 1], dt.uint16)
        b1t = const.tile([128, 1], dt.float32)
        b2t = const.tile([1, 1], dt.float32)
        zsrc = const.tile([128, E_LOC], dt.uint16)
        zdst = const.tile([128, E_LOC], dt.uint16)

        st_state = {}

        def load(et, eng=None):
            n = STS[et]
            rs = rows.tile([128, W_TOT * n], dt.uint16, tag="rs", name=f"rs{et}")
            rd = rows.tile([128, W_TOT * n], dt.uint16, tag="rd", name=f"rd{et}")
            c0, c1 = W_TOT * OFFS[et], W_TOT * OFFS[et + 1]
            (eng or nc.sync).dma_start(rs[:], t["RS"][:, c0:c1])
            (eng or nc.sync).dma_start(rd[:], t["RD"][:, c0:c1])
            st_state[et] = (rs, rd)

        def load_stationary_chunk(b):
            nc.sync.dma_start(
                zds[:, b * C_A * 256:(b + 1) * C_A * 256],
                t["ZDS"][:, b * C_A * 256:(b + 1) * C_A * 256],
            )
            nc.sync.dma_start(
                zps[:, b * C_P * 256:(b + 1) * C_P * 256],
                t["ZPS"][:, b * C_P * 256:(b + 1) * C_P * 256],
            )

        # Priority-ordered prologue loads: first supertile rows, then the
        # first stationary chunk, MLP weights, next rows, remaining chunks.
        # The first supertile is split into two w-halves on separate tiles
        # across both HWDGE queues (scalar's sequencer boots ~2.3us before
        # sync's), so its AND/extract/matmuls start after only half the
        # bytes have landed.
        def load0():
            n = STS[0]
            h = W_TOT * n // 2
            tiles = []
            for half, eng, c0 in (("a", nc.scalar, 0), ("b", nc.sync, h)):
                rs = rows.tile([128, h], dt.uint16, tag=f"rs0{half}", bufs=1)
                rd = rows.tile([128, h], dt.uint16, tag=f"rd0{half}", bufs=1)
                eng.dma_start(rs[:], t["RS"][:, c0:c0 + h])
                eng.dma_start(rd[:], t["RD"][:, c0:c0 + h])
                tiles.append((rs, rd))
            st_state[0] = tiles

        load0()
        load_stationary_chunk(0)
        nc.sync.dma_start(w1t[:], t["W1"][:, :])
        nc.sync.dma_start(b1t[:], t["B1"][:, :])
        nc.sync.dma_start(w2t[:], t["W2"][:, :])
        nc.sync.dma_start(b2t[:], t["B2"][:, :])
        load(1)
        for b in range(1, 8):
            load_stationary_chunk(b)
        nc.sync.dma_start(zsrc[:], t["ZS"][:, :])
        nc.sync.dma_start(zdst[:], t["ZD"][:, :])



        zds8 = zds[:].bitcast(dt.float8e4).rearrange(
            "p (g two c) -> p g two c", g=G_A, two=2
        )
        zps8 = zps[:].bitcast(dt.float8e4).rearrange(
            "p (g two c) -> p g two c", g=G_P, two=2
        )

        def extract(pl, at, b):
            """Fused shift+mask: bits (b, 8+b) -> fp8 0x08 at bytes (0, 1)."""
            if b < 3:
                nc.vector.tensor_scalar(
                    pl[:], at[:], 3 - b, 0x0808,
                    mybir.AluOpType.logical_shift_left,
                    mybir.AluOpType.bitwise_and,
                )
            elif b == 3:
                nc.vector.tensor_scalar(
                    pl[:], at[:], 0x0808, None,
                    mybir.AluOpType.bitwise_and,
                )
            else:
                nc.vector.tensor_scalar(
                    pl[:], at[:], b - 3, 0x0808,
                    mybir.AluOpType.logical_shift_right,
                    mybir.AluOpType.bitwise_and,
                )

        def and_(et):
            if et == 0:
                ats = []
                for half, (rs, rd) in zip("ab", st_state[0]):
                    at = andp.tile([128, W_TOT * STS[0] // 2], dt.uint16,
                                   tag=f"and0{half}", bufs=1)
                    nc.vector.tensor_tensor(
                        at[:], rs[:], rd[:], mybir.AluOpType.bitwise_and
                    )
                    ats.append(at)
                st_state[0] = ats
                return
            n = STS[et]
            rs, rd = st_state[et]
            at = andp.tile([128, W_TOT * n], dt.uint16, tag="and", name=f"and{et}")
            nc.vector.tensor_tensor(
                at[:], rs[:], rd[:], mybir.AluOpType.bitwise_and
            )
            st_state[et] = (rs, rd, at)

        def planes_mm(et):
            n = STS[et]
            _, _, at = st_state[et]
            psa = ps_cn.tile([128, n], dt.float32, tag="psa", name=f"psa{et}")
            psb = ps_cn.tile([128, n], dt.float32, tag="psb", name=f"psb{et}")
            for b in range(8):
                pl = plnp.tile([128, W_TOT * n], dt.uint16, tag="pl",
                               name=f"pl{et}_{b}")
                extract(pl, at, b)
                mv = pl[:].bitcast(dt.float8e4).rearrange(
                    "p (w e two) -> p w two e", w=W_TOT, two=2
                )
                for C in range(C_A):
                    nc.tensor.matmul(
                        psa[:],
                        zds8[:, b * C_A + C],
                        mv[:, C],
                        start=(b == 0 and C == 0),
                        stop=(b == 7 and C == C_A - 1),
                        perf_mode=mybir.MatmulPerfMode.DoubleRow,
                    )
                for C in range(C_P):
                    nc.tensor.matmul(
                        psb[:],
                        zps8[:, b * C_P + C],
                        mv[:, C_A + C],
                        start=(b == 0 and C == 0),
                        stop=(b == 7 and C == C_P - 1),
                        perf_mode=mybir.MatmulPerfMode.DoubleRow,
                    )
            st_state[et] = (psa, psb)

        def planes_mm0():
            """First supertile: per-b extraction + matmuls run on two
            w-halves so the PE starts after only half the rows arrived."""
            n = STS[0]
            W_H = W_TOT // 2
            ata, atb = st_state[0]
            psa = ps_cn.tile([128, n], dt.float32, tag="psa", name="psa0")
            psb = ps_cn.tile([128, n], dt.float32, tag="psb", name="psb0")
            for b in range(8):
                for hi, at in ((0, ata), (1, atb)):
                    pl = plnp.tile([128, W_H * n], dt.uint16, tag="pl0",
                                   name=f"pl0_{b}_{hi}")
                    extract(pl, at, b)
                    mv = pl[:].bitcast(dt.float8e4).rearrange(
                        "p (w e two) -> p w two e", w=W_H, two=2
                    )
                    for wloc in range(W_H):
                        w = hi * W_H + wloc
                        if w < C_A:
                            nc.tensor.matmul(
                                psa[:],
                                zds8[:, b * C_A + w],
                                mv[:, wloc],
                                start=(b == 0 and w == 0),
                                stop=(b == 7 and w == C_A - 1),
                                perf_mode=mybir.MatmulPerfMode.DoubleRow,
                            )
                        else:
                            Cp = w - C_A
                            nc.tensor.matmul(
                                psb[:],
                                zps8[:, b * C_P + Cp],
                                mv[:, wloc],
                                start=(b == 0 and Cp == 0),
                                stop=(b == 7 and Cp == C_P - 1),
                                perf_mode=mybir.MatmulPerfMode.DoubleRow,
                            )
            st_state[0] = (psa, psb)

        def tail(et):
            """Normalize + MLP + output for supertile et. The scalar engine
            moves the PSUM row-0 counts out with a +eps bias folded into an
            identity activation, so no clamp is needed: rec = 1/(cnt + eps).
            Count-0 edges have exactly-zero psum rows, so 0 * (1/eps) = 0
            matches the reference's max(cnt, 1); for cnt = n*S_OUT the
            relative bias is 2^-12/n."""
            n = STS[et]
            eps = float(S_OUT * 2.0 ** -12)
            psa, psb = st_state.pop(et)
            cnt = tails.tile([1, 2 * n], dt.float32, tag="cnt", bufs=1)
            nc.scalar.activation(
                cnt[0:1, 0:n], psa[0:1, :],
                mybir.ActivationFunctionType.Copy, bias=eps,
            )
            nc.scalar.activation(
                cnt[0:1, n:2 * n], psb[0:1, :],
                mybir.ActivationFunctionType.Copy, bias=eps,
            )
            rec = tails.tile([1, 2 * n], dt.float32, tag="rec", bufs=1)
            nc.vector.reciprocal_approx_fast(rec[:], cnt[:])
            rec_bf = tails.tile([1, 2 * n], dt.bfloat16, tag="recb", bufs=1)
            nc.scalar.copy(rec_bf[:], rec[:])
            rhs = {}
            for rel, ps, c0 in (("a", psa, 0), ("b", psb, n)):
                bcs = tails.tile([128, n], dt.bfloat16, tag=f"bc{rel}", bufs=1)
                nc.gpsimd.partition_broadcast(bcs[:], rec_bf[0:1, c0:c0 + n])
                pss = tails.tile([128, n], dt.bfloat16, tag=f"ps{rel}", bufs=1)
                nc.scalar.copy(pss[:], ps[:])
                rh = tails.tile([128, n], dt.bfloat16, tag=f"rh{rel}")
                nc.vector.tensor_tensor(
                    rh[:], pss[:], bcs[:], mybir.AluOpType.mult
                )
                rhs[rel] = rh
            hps = ps_ml.tile([HID, n], dt.float32, tag="ps", bufs=1)
            rhs_chunks = (
                zsrc[:].bitcast(dt.bfloat16)[:, OFFS[et]:OFFS[et + 1]],
                zdst[:].bitcast(dt.bfloat16)[:, OFFS[et]:OFFS[et + 1]],
                rhs["a"][:],
                rhs["b"][:],
            )
            for j, r in enumerate(rhs_chunks):
                nc.tensor.matmul(
                    hps[:],
                    w1t[:].bitcast(dt.bfloat16)[:, HID * j:HID * (j + 1)],
                    r,
                    start=(j == 0),
                    stop=(j == 3),
                )
            hsb = tails.tile([HID, n], dt.bfloat16, tag="h", bufs=1)
            nc.scalar.activation(
                hsb[:], hps[:], mybir.ActivationFunctionType.Relu, bias=b1t[:, 0:1]
            )
            lps = ps_ml.tile([1, n], dt.float32, tag="lps", bufs=1)
            nc.tensor.matmul(
                lps[:], w2t[:].bitcast(dt.bfloat16), hsb[:], start=True, stop=True
            )
            osb = tails.tile([1, n], dt.float32, tag="osb", bufs=1)
            nc.scalar.activation(
                osb[:],
                lps[:],
                mybir.ActivationFunctionType.Sigmoid,
                bias=b2t[:, 0:1],
            )
            nc.sync.dma_start(t["OUT"][:, OFFS[et]:OFFS[et + 1]], osb[:])

        # Emission order per iteration keeps every engine queue stall-free:
        # DVE sees [AND(et), cnt+recip(et-1), extracts(et), mults(et-1)] —
        # the tail chain never sits in front of plane production, and the
        # PE sees [cn(et), MLP(et-1)] whose rhs is long ready.
        for et in range(N_ET):
            if et + 2 < N_ET:
                load(et + 2)
            and_(et)
            if et == 0:
                planes_mm0()
            else:
                planes_mm(et)
            if et > 0:
                tail(et - 1)
        tail(N_ET - 1)


def build_program():
    nc = bacc.Bacc("TRN2", target_bir_lowering=False)
    dt = mybir.dt
    t = {
        "RS": nc.dram_tensor(
            "RS", [128, W_TOT * E_LOC], dt.uint16, kind="ExternalInput"
        ).ap(),
        "RD": nc.dram_tensor(
            "RD", [128, W_TOT * E_LOC], dt.uint16, kind="ExternalInput"
        ).ap(),
        "ZS": nc.dram_tensor("ZS", [128, E_LOC], dt.uint16, kind="ExternalInput").ap(),
        "ZD": nc.dram_tensor("ZD", [128, E_LOC], dt.uint16, kind="ExternalInput").ap(),
        "ZDS": nc.dram_tensor(
            "ZDS", [128, G_A * 2 * 128], dt.uint8, kind="ExternalInput"
        ).ap(),
        "ZPS": nc.dram_tensor(
            "ZPS", [128, G_P * 2 * 128], dt.uint8, kind="ExternalInput"
        ).ap(),
        "W1": nc.dram_tensor("W1", [128, 4 * HID], dt.uint16, kind="ExternalInput").ap(),
        "B1": nc.dram_tensor("B1", [HID, 1], dt.float32, kind="ExternalInput").ap(),
        "W2": nc.dram_tensor("W2", [HID, 1], dt.uint16, kind="ExternalInput").ap(),
        "B2": nc.dram_tensor("B2", [1, 1], dt.float32, kind="ExternalInput").ap(),
        "OUT": nc.dram_tensor("OUT", [1, E_LOC], dt.float32, kind="ExternalOutput").ap(),
    }
    with tile.TileContext(nc) as tc:
        build_body(tc, t)
    nc.compile()
    return nc


def host_prep(z_drug, z_protein, ddi_ei, dp_ei, pred_ei, W1, b1, W2, b2):
    """Build the 8 per-core input maps (all numpy, no device work)."""
    z_drug = np.asarray(z_drug, np.float32)
    z_protein = np.asarray(z_protein, np.float32)
    ddi_ei = np.asarray(ddi_ei, np.int64)
    dp_ei = np.asarray(dp_ei, np.int64)
    pred_ei = np.asarray(pred_ei, np.int64)

    A_ddi = np.zeros((N_D, N_D), dtype=np.uint8)
    A_ddi[ddi_ei[0], ddi_ei[1]] = 1
    A_ddi[ddi_ei[1], ddi_ei[0]] = 1
    A_dp = np.zeros((N_D, N_P), dtype=np.uint8)
    A_dp[dp_ei[0], dp_ei[1]] = 1

    # 1-bit packed, pre-transposed row content: word (row, w, p) holds
    # neighbor j = 2048*w + 16*p + beta at bit beta (little-endian).
    P_ddi = np.packbits(
        A_ddi.reshape(N_D, C_A, 128, 16), axis=-1, bitorder="little"
    ).view(np.uint16)[..., 0]
    P_dp = np.packbits(
        A_dp.reshape(N_D, C_P, 128, 16), axis=-1, bitorder="little"
    ).view(np.uint16)[..., 0]
    P_all = np.concatenate([P_ddi, P_dp], axis=1)       # [N_D, 6, 128]

    zb = np.zeros((128, N_D), dtype=np.uint16)
    zb[0:D_DIM] = z_drug.astype(BF16).view(np.uint16).T

    ZDS = _pack_stationary(z_drug)
    ZPS = _pack_stationary(z_protein)

    # W1 blocks [128, HID] bf16: block0/1 rows 0:64 = z dims; block2/3 row 0
    # = 0 (count row), rows 1:65 = hi-limb dims, 65:128 = lo-limb dims 0..62
    # (duplicated weights sum the limbs).
    W1f = np.asarray(W1, np.float32)  # [256, HID]
    blocks = np.zeros((4, 128, HID), dtype=np.float32)
    blocks[0, 0:64] = W1f[0:64]
    blocks[1, 0:64] = W1f[64:128]
    blocks[2, 1:65] = W1f[128:192]
    blocks[2, 65:128] = W1f[128:191]
    blocks[3, 1:65] = W1f[192:256]
    blocks[3, 65:128] = W1f[192:255]
    W1p = np.ascontiguousarray(
        blocks.astype(BF16).view(np.uint16).transpose(1, 0, 2).reshape(128, 4 * HID)
    )
    B1 = np.asarray(b1, np.float32).reshape(HID, 1)
    W2p = np.asarray(W2, np.float32).reshape(HID, 1).astype(BF16).view(np.uint16)
    B2 = np.asarray(b2, np.float32).reshape(1, 1)

    in_maps = []
    for core in range(N_CORES):
        s = pred_ei[0, core * E_LOC:(core + 1) * E_LOC]
        d = pred_ei[1, core * E_LOC:(core + 1) * E_LOC]
        # per-supertile [n, 6, 128] -> [128, w, e] blocks, concatenated
        RS = np.concatenate(
            [
                P_all[s[OFFS[et]:OFFS[et + 1]]].transpose(2, 1, 0).reshape(128, -1)
                for et in range(N_ET)
            ],
            axis=1,
        )
        RD = np.concatenate(
            [
                P_all[d[OFFS[et]:OFFS[et + 1]]].transpose(2, 1, 0).reshape(128, -1)
                for et in range(N_ET)
            ],
            axis=1,
        )
        in_maps.append(
            {
                "RS": np.ascontiguousarray(RS),
                "RD": np.ascontiguousarray(RD),
                "ZS": np.ascontiguousarray(zb[:, s]),
                "ZD": np.ascontiguousarray(zb[:, d]),
                "ZDS": ZDS,
                "ZPS": ZPS,
                "W1": W1p,
                "B1": B1,
                "W2": W2p,
                "B2": B2,
            }
        )
    return in_maps


def kernel(z_drug, z_protein, ddi_ei, dp_ei, pred_ei, W1, b1, W2, b2, _profile=None):
    from concourse.bass_utils import run_bass_kernel_spmd

    in_maps = host_prep(z_drug, z_protein, ddi_ei, dp_ei, pred_ei, W1, b1, W2, b2)
    nc = build_program()
    res = run_bass_kernel_spmd(
        nc,
        in_maps,
        core_ids=list(range(N_CORES)),
        **({} if _profile is None else _profile),
    )
    if _profile is not None:
        kernel.last_results = res
    out = np.concatenate([r["OUT"].reshape(-1) for r in res.results])
    return out.astype(np.float32)


# revision 43
# speedup vs baseline: 1.0085x; 1.0085x over previous
"""Trainium2 Bass kernel for EnhancedLinkPredictor (GNN common-neighbor link prediction).

Math (per prediction edge e=(s,d)):
  shared_ddi = adj_ddi[s] & adj_ddi[d]          (drug-drug, N_D=8192)
  cn_ddi     = (shared_ddi @ z_drug)  / max(|shared_ddi|, 1)
  shared_dp  = adj_dp[s]  & adj_dp[d]           (drug-protein, N_P=4096)
  cn_prot    = (shared_dp @ z_protein) / max(|shared_dp|, 1)
  pair  = [z_drug[s], z_drug[d], cn_ddi, cn_prot]   (256)
  out   = sigmoid(relu(pair @ W1 + b1) @ W2 + b2)

Device strategy (8 cores, data-parallel over the 16384 pred edges, 2048/core):
  - No on-device gathers. pred_ei is host-known, so the host materializes the
    s-side and d-side adjacency rows in EDGE ORDER, 1-bit packed and
    pre-transposed to the SBUF layout: partition p's u16 word for superchunk C
    holds neighbors j = 2048*C + 16*p + 8*i + b at bit (8*i + b). Rows are
    streamed with plain HWDGE dma_start on one priority-ordered queue.
  - Device AND: one tensor_tensor(bitwise_and) per supertile computes s&d
    over the packed bits ([128, 6 words, E] u16).
  - Bit->fp8 expansion: for each bit-pair b (8 planes/supertile), ONE fused
    tensor_scalar((and <<|>> sh_b) & 0x0808) produces the fp8 e4m3 plane
    directly (byte 0x08 = 2^-6); bits (b, 8+b) land at byte positions (0, 1)
    = the DoubleRow 'two' sub-rows. Runs in the DVE 4x perf mode.
  - cn matmuls: fp8 DoubleRow, stationary [128, 2, 128] holding
    [count | hi limb of 4*z (64) | lo limb dims 0..62]; hi+lo summation
    happens inside the MLP W1 matmul via duplicated W1 rows. Stationaries
    are laid out b-major and DMA'd in per-b chunks so the first matmuls'
    weights arrive first.
  - Normalize: counts sit in PSUM row 0; DVE max+reciprocal on [1, 2E],
    gpsimd broadcasts 1/cnt to 128 partitions and also does the psum*rec
    multiplies (keeping the DVE free for mask planes).
  - Supertile sizes (256, 512, 512, 512, 256): small first tile starts the
    PE early; small last tile shortens the serial epilogue tail.
"""

import numpy as np
import ml_dtypes
from contextlib import ExitStack

import concourse.bass as bass
import concourse.bacc as bacc
import concourse.mybir as mybir
import concourse.tile as tile

N_D, N_P = 8192, 4096
D_DIM, HID = 64, 128
E_PRED = 16384
N_CORES = 8
E_LOC = E_PRED // N_CORES          # 2048 edges per core
STS = (512, 512, 512, 512)         # supertile sizes
OFFS = tuple(int(x) for x in np.cumsum((0,) + STS))  # edge offsets
N_ET = len(STS)
C_A = N_D // 2048                  # 4 ddi superchunks (2048 entries each)
C_P = N_P // 2048                  # 2 dp superchunks
W_TOT = C_A + C_P                  # 6 u16 words per partition per row
G_A = C_A * 8                      # 32 ddi stationary groups
G_P = C_P * 8                      # 16 dp stationary groups

S_OUT = 2.0 ** -4                  # plane code (2^-6) * z premult (4) product
Z_SCALE = 4.0

FP8 = ml_dtypes.float8_e4m3
BF16 = ml_dtypes.bfloat16


def _pack_stationary(z: np.ndarray):
    """z [K, 64] f32 -> [128, (K/2048)*8*2*128] uint8 fp8 DoubleRow lhsT.
    B-MAJOR group order: g = b*(K/2048) + C holds rows k = 2048C + 16p + 8i + b
    at (partition p, sub-row i), scaled by Z_SCALE. Cols: 0 = count (Z_SCALE),
    [1:65] hi limb, [65:128] lo limb of dims 0..62."""
    K = z.shape[0]
    nC = K // 2048
    p = np.arange(128)[:, None]
    i = np.arange(2)[None, :]
    out = np.empty((8 * nC, 128, 2, 128), dtype=np.uint8)
    code = np.float32(Z_SCALE).astype(FP8).view(np.uint8)
    for C in range(nC):
        for b in range(8):
            ks = 2048 * C + 16 * p + 8 * i + b            # [128, 2]
            zsc = z[ks].astype(np.float32) * Z_SCALE      # [128, 2, 64]
            hi8 = zsc.astype(FP8)
            lo8 = (zsc - hi8.astype(np.float32)).astype(FP8)
            blk = np.zeros((128, 2, 128), dtype=np.uint8)
            blk[..., 0] = code
            blk[..., 1:65] = hi8.view(np.uint8)
            blk[..., 65:128] = lo8.view(np.uint8)[..., :63]
            out[b * nC + C] = blk
    return np.ascontiguousarray(out.transpose(1, 0, 2, 3).reshape(128, -1))


def build_body(tc, t):
    """Emit the per-core program. t: dict name -> AP of DRAM tensors."""
    nc = tc.nc
    dt = mybir.dt
    with ExitStack() as ctx:
        const = ctx.enter_context(tc.tile_pool(name="const", bufs=1))
        rows = ctx.enter_context(tc.tile_pool(name="rows", bufs=3))
        andp = ctx.enter_context(tc.tile_pool(name="andp", bufs=2))
        plnp = ctx.enter_context(tc.tile_pool(name="plnp", bufs=10))
        tails = ctx.enter_context(tc.tile_pool(name="tails", bufs=2))
        ps_cn = ctx.enter_context(tc.tile_pool(name="ps_cn", bufs=2, space="PSUM"))
        ps_ml = ctx.enter_context(tc.tile_pool(name="ps_ml", bufs=2, space="PSUM"))

        # All input loads on the single scalar HWDGE queue, in need-order
        # (strict priority; its sequencer also boots earliest). OUT stores
        # go on the sync queue.
        zds = const.tile([128, G_A * 2 * 128], dt.uint8)
        zps = const.tile([128, G_P * 2 * 128], dt.uint8)
        w1t = const.tile([128, 4 * HID], dt.uint16)
        w2t = const.tile([128, 1], dt.uint16)
        b1t = const.tile([128, 1], dt.float32)
        b2t = const.tile([1, 1], dt.float32)
        zsrc = const.tile([128, E_LOC], dt.uint16)
        zdst = const.tile([128, E_LOC], dt.uint16)

        st_state = {}

        def load(et, eng=None):
            n = STS[et]
            rs = rows.tile([128, W_TOT * n], dt.uint16, tag="rs", name=f"rs{et}")
            rd = rows.tile([128, W_TOT * n], dt.uint16, tag="rd", name=f"rd{et}")
            c0, c1 = W_TOT * OFFS[et], W_TOT * OFFS[et + 1]
            (eng or nc.sync).dma_start(rs[:], t["RS"][:, c0:c1])
            (eng or nc.sync).dma_start(rd[:], t["RD"][:, c0:c1])
            st_state[et] = (rs, rd)

        def load_stationary_chunk(b):
            nc.scalar.dma_start(
                zds[:, b * C_A * 256:(b + 1) * C_A * 256],
                t["ZDS"][:, b * C_A * 256:(b + 1) * C_A * 256],
            )
            nc.scalar.dma_start(
                zps[:, b * C_P * 256:(b + 1) * C_P * 256],
                t["ZPS"][:, b * C_P * 256:(b + 1) * C_P * 256],
            )

        # Priority-ordered prologue loads: first supertile rows, then the
        # first stationary chunk, MLP weights, next rows, remaining chunks.
        # The first rows go out on the scalar HWDGE queue — the ACT
        # sequencer boots ~2.3us before the sync one.
        load(0, eng=nc.scalar)
        load_stationary_chunk(0)
        nc.scalar.dma_start(w1t[:], t["W1"][:, :])
        nc.scalar.dma_start(b1t[:], t["B1"][:, :])
        nc.scalar.dma_start(w2t[:], t["W2"][:, :])
        nc.scalar.dma_start(b2t[:], t["B2"][:, :])
        load(1, eng=nc.scalar)
        for b in range(1, 8):
            load_stationary_chunk(b)
        nc.scalar.dma_start(zsrc[:], t["ZS"][:, :])
        nc.scalar.dma_start(zdst[:], t["ZD"][:, :])



        zds8 = zds[:].bitcast(dt.float8e4).rearrange(
            "p (g two c) -> p g two c", g=G_A, two=2
        )
        zps8 = zps[:].bitcast(dt.float8e4).rearrange(
            "p (g two c) -> p g two c", g=G_P, two=2
        )

        def and_(et):
            n = STS[et]
            rs, rd = st_state[et]
            at = andp.tile([128, W_TOT * n], dt.uint16, tag="and", name=f"and{et}")
            nc.vector.tensor_tensor(
                at[:], rs[:], rd[:], mybir.AluOpType.bitwise_and
            )
            st_state[et] = (rs, rd, at)

        def planes_mm(et):
            n = STS[et]
            _, _, at = st_state[et]
            psa = ps_cn.tile([128, n], dt.float32, tag="psa", name=f"psa{et}")
            psb = ps_cn.tile([128, n], dt.float32, tag="psb", name=f"psb{et}")
            for b in range(8):
                pl = plnp.tile([128, W_TOT * n], dt.uint16, tag="pl",
                               name=f"pl{et}_{b}")
                if b < 3:
                    nc.vector.tensor_scalar(
                        pl[:], at[:], 3 - b, 0x0808,
                        mybir.AluOpType.logical_shift_left,
                        mybir.AluOpType.bitwise_and,
                    )
                elif b == 3:
                    nc.vector.tensor_scalar(
                        pl[:], at[:], 0x0808, None,
                        mybir.AluOpType.bitwise_and,
                    )
                else:
                    nc.vector.tensor_scalar(
                        pl[:], at[:], b - 3, 0x0808,
                        mybir.AluOpType.logical_shift_right,
                        mybir.AluOpType.bitwise_and,
                    )
                mv = pl[:].bitcast(dt.float8e4).rearrange(
                    "p (w e two) -> p w two e", w=W_TOT, two=2
                )
                for C in range(C_A):
                    nc.tensor.matmul(
                        psa[:],
                        zds8[:, b * C_A + C],
                        mv[:, C],
                        start=(b == 0 and C == 0),
                        stop=(b == 7 and C == C_A - 1),
                        perf_mode=mybir.MatmulPerfMode.DoubleRow,
                    )
                for C in range(C_P):
                    nc.tensor.matmul(
                        psb[:],
                        zps8[:, b * C_P + C],
                        mv[:, C_A + C],
                        start=(b == 0 and C == 0),
                        stop=(b == 7 and C == C_P - 1),
                        perf_mode=mybir.MatmulPerfMode.DoubleRow,
                    )
            st_state[et] = (psa, psb)

        def tail(et):
            """Normalize + MLP + output for supertile et. The scalar engine
            moves the PSUM row-0 counts out with a +eps bias folded into an
            identity activation, so no clamp is needed: rec = 1/(cnt + eps).
            Count-0 edges have exactly-zero psum rows, so 0 * (1/eps) = 0
            matches the reference's max(cnt, 1); for cnt = n*S_OUT the
            relative bias is 2^-12/n."""
            n = STS[et]
            eps = float(S_OUT * 2.0 ** -12)
            psa, psb = st_state.pop(et)
            cnt = tails.tile([1, 2 * n], dt.float32, tag="cnt", bufs=1)
            nc.scalar.activation(
                cnt[0:1, 0:n], psa[0:1, :],
                mybir.ActivationFunctionType.Copy, bias=eps,
            )
            nc.scalar.activation(
                cnt[0:1, n:2 * n], psb[0:1, :],
                mybir.ActivationFunctionType.Copy, bias=eps,
            )
            rec = tails.tile([1, 2 * n], dt.float32, tag="rec", bufs=1)
            nc.vector.reciprocal_approx_fast(rec[:], cnt[:])
            rec_bf = tails.tile([1, 2 * n], dt.bfloat16, tag="recb", bufs=1)
            nc.scalar.copy(rec_bf[:], rec[:])
            rhs = {}
            for rel, ps, c0 in (("a", psa, 0), ("b", psb, n)):
                bcs = tails.tile([128, n], dt.bfloat16, tag=f"bc{rel}", bufs=1)
                nc.gpsimd.partition_broadcast(bcs[:], rec_bf[0:1, c0:c0 + n])
                pss = tails.tile([128, n], dt.bfloat16, tag=f"ps{rel}", bufs=1)
                nc.scalar.copy(pss[:], ps[:])
                rh = tails.tile([128, n], dt.bfloat16, tag=f"rh{rel}")
                nc.vector.tensor_tensor(
                    rh[:], pss[:], bcs[:], mybir.AluOpType.mult
                )
                rhs[rel] = rh
            hps = ps_ml.tile([HID, n], dt.float32, tag="ps", bufs=1)
            rhs_chunks = (
                zsrc[:].bitcast(dt.bfloat16)[:, OFFS[et]:OFFS[et + 1]],
                zdst[:].bitcast(dt.bfloat16)[:, OFFS[et]:OFFS[et + 1]],
                rhs["a"][:],
                rhs["b"][:],
            )
            for j, r in enumerate(rhs_chunks):
                nc.tensor.matmul(
                    hps[:],
                    w1t[:].bitcast(dt.bfloat16)[:, HID * j:HID * (j + 1)],
                    r,
                    start=(j == 0),
                    stop=(j == 3),
                )
            hsb = tails.tile([HID, n], dt.bfloat16, tag="h", bufs=1)
            nc.scalar.activation(
                hsb[:], hps[:], mybir.ActivationFunctionType.Relu, bias=b1t[:, 0:1]
            )
            lps = ps_ml.tile([1, n], dt.float32, tag="lps", bufs=1)
            nc.tensor.matmul(
                lps[:], w2t[:].bitcast(dt.bfloat16), hsb[:], start=True, stop=True
            )
            osb = tails.tile([1, n], dt.float32, tag="osb", bufs=1)
            nc.scalar.activation(
                osb[:],
                lps[:],
                mybir.ActivationFunctionType.Sigmoid,
                bias=b2t[:, 0:1],
            )
            nc.sync.dma_start(t["OUT"][:, OFFS[et]:OFFS[et + 1]], osb[:])

        # Emission order per iteration keeps every engine queue stall-free:
        # DVE sees [AND(et), cnt+recip(et-1), extracts(et), mults(et-1)] —
        # the tail chain never sits in front of plane production, and the
        # PE sees [cn(et), MLP(et-1)] whose rhs is long ready.
        for et in range(N_ET):
            if et + 2 < N_ET:
                load(et + 2, eng=nc.scalar)
            and_(et)
            planes_mm(et)
            if et > 0:
                tail(et - 1)
        tail(N_ET - 1)


def build_program():
    nc = bacc.Bacc("TRN2", target_bir_lowering=False)
    dt = mybir.dt
    t = {
        "RS": nc.dram_tensor(
            "RS", [128, W_TOT * E_LOC], dt.uint16, kind="ExternalInput"
        ).ap(),
        "RD": nc.dram_tensor(
            "RD", [128, W_TOT * E_LOC], dt.uint16, kind="ExternalInput"
        ).ap(),
        "ZS": nc.dram_tensor("ZS", [128, E_LOC], dt.uint16, kind="ExternalInput").ap(),
        "ZD": nc.dram_tensor("ZD", [128, E_LOC], dt.uint16, kind="ExternalInput").ap(),
        "ZDS": nc.dram_tensor(
            "ZDS", [128, G_A * 2 * 128], dt.uint8, kind="ExternalInput"
        ).ap(),
        "ZPS": nc.dram_tensor(
            "ZPS", [128, G_P * 2 * 128], dt.uint8, kind="ExternalInput"
        ).ap(),
        "W1": nc.dram_tensor("W1", [128, 4 * HID], dt.uint16, kind="ExternalInput").ap(),
        "B1": nc.dram_tensor("B1", [HID, 1], dt.float32, kind="ExternalInput").ap(),
        "W2": nc.dram_tensor("W2", [HID, 1], dt.uint16, kind="ExternalInput").ap(),
        "B2": nc.dram_tensor("B2", [1, 1], dt.float32, kind="ExternalInput").ap(),
        "OUT": nc.dram_tensor("OUT", [1, E_LOC], dt.float32, kind="ExternalOutput").ap(),
    }
    with tile.TileContext(nc) as tc:
        build_body(tc, t)
    nc.compile()
    return nc


def host_prep(z_drug, z_protein, ddi_ei, dp_ei, pred_ei, W1, b1, W2, b2):
    """Build the 8 per-core input maps (all numpy, no device work)."""
    z_drug = np.asarray(z_drug, np.float32)
    z_protein = np.asarray(z_protein, np.float32)
    ddi_ei = np.asarray(ddi_ei, np.int64)
    dp_ei = np.asarray(dp_ei, np.int64)
    pred_ei = np.asarray(pred_ei, np.int64)

    A_ddi = np.zeros((N_D, N_D), dtype=np.uint8)
    A_ddi[ddi_ei[0], ddi_ei[1]] = 1
    A_ddi[ddi_ei[1], ddi_ei[0]] = 1
    A_dp = np.zeros((N_D, N_P), dtype=np.uint8)
    A_dp[dp_ei[0], dp_ei[1]] = 1

    # 1-bit packed, pre-transposed row content: word (row, w, p) holds
    # neighbor j = 2048*w + 16*p + beta at bit beta (little-endian).
    P_ddi = np.packbits(
        A_ddi.reshape(N_D, C_A, 128, 16), axis=-1, bitorder="little"
    ).view(np.uint16)[..., 0]
    P_dp = np.packbits(
        A_dp.reshape(N_D, C_P, 128, 16), axis=-1, bitorder="little"
    ).view(np.uint16)[..., 0]
    P_all = np.concatenate([P_ddi, P_dp], axis=1)       # [N_D, 6, 128]

    zb = np.zeros((128, N_D), dtype=np.uint16)
    zb[0:D_DIM] = z_drug.astype(BF16).view(np.uint16).T

    ZDS = _pack_stationary(z_drug)
    ZPS = _pack_stationary(z_protein)

    # W1 blocks [128, HID] bf16: block0/1 rows 0:64 = z dims; block2/3 row 0
    # = 0 (count row), rows 1:65 = hi-limb dims, 65:128 = lo-limb dims 0..62
    # (duplicated weights sum the limbs).
    W1f = np.asarray(W1, np.float32)  # [256, HID]
    blocks = np.zeros((4, 128, HID), dtype=np.float32)
    blocks[0, 0:64] = W1f[0:64]
    blocks[1, 0:64] = W1f[64:128]
    blocks[2, 1:65] = W1f[128:192]
    blocks[2, 65:128] = W1f[128:191]
    blocks[3, 1:65] = W1f[192:256]
    blocks[3, 65:128] = W1f[192:255]
    W1p = np.ascontiguousarray(
        blocks.astype(BF16).view(np.uint16).transpose(1, 0, 2).reshape(128, 4 * HID)
    )
    B1 = np.asarray(b1, np.float32).reshape(HID, 1)
    W2p = np.asarray(W2, np.float32).reshape(HID, 1).astype(BF16).view(np.uint16)
    B2 = np.asarray(b2, np.float32).reshape(1, 1)

    in_maps = []
    for core in range(N_CORES):
        s = pred_ei[0, core * E_LOC:(core + 1) * E_LOC]
        d = pred_ei[1, core * E_LOC:(core + 1) * E_LOC]
        # per-supertile [n, 6, 128] -> [128, w, e] blocks, concatenated
        RS = np.concatenate(
            [
                P_all[s[OFFS[et]:OFFS[et + 1]]].transpose(2, 1, 0).reshape(128, -1)
                for et in range(N_ET)
            ],
            axis=1,
        )
        RD = np.concatenate(
            [
                P_all[d[OFFS[et]:OFFS[et + 1]]].transpose(2, 1, 0).reshape(128, -1)
                for et in range(N_ET)
            ],
            axis=1,
        )
        in_maps.append(
            {
                "RS": np.ascontiguousarray(RS),
                "RD": np.ascontiguousarray(RD),
                "ZS": np.ascontiguousarray(zb[:, s]),
                "ZD": np.ascontiguousarray(zb[:, d]),
                "ZDS": ZDS,
                "ZPS": ZPS,
                "W1": W1p,
                "B1": B1,
                "W2": W2p,
                "B2": B2,
            }
        )
    return in_maps


def kernel(z_drug, z_protein, ddi_ei, dp_ei, pred_ei, W1, b1, W2, b2, _profile=None):
    from concourse.bass_utils import run_bass_kernel_spmd

    in_maps = host_prep(z_drug, z_protein, ddi_ei, dp_ei, pred_ei, W1, b1, W2, b2)
    nc = build_program()
    res = run_bass_kernel_spmd(
        nc,
        in_maps,
        core_ids=list(range(N_CORES)),
        **({} if _profile is None else _profile),
    )
    if _profile is not None:
        kernel.last_results = res
    out = np.concatenate([r["OUT"].reshape(-1) for r in res.results])
    return out.astype(np.float32)


# revision 45
# speedup vs baseline: 1.1141x; 1.1047x over previous
"""Trainium2 Bass kernel for EnhancedLinkPredictor (GNN common-neighbor link prediction).

Math (per prediction edge e=(s,d)):
  shared_ddi = adj_ddi[s] & adj_ddi[d]          (drug-drug, N_D=8192)
  cn_ddi     = (shared_ddi @ z_drug)  / max(|shared_ddi|, 1)
  shared_dp  = adj_dp[s]  & adj_dp[d]           (drug-protein, N_P=4096)
  cn_prot    = (shared_dp @ z_protein) / max(|shared_dp|, 1)
  pair  = [z_drug[s], z_drug[d], cn_ddi, cn_prot]   (256)
  out   = sigmoid(relu(pair @ W1 + b1) @ W2 + b2)

Device strategy (8 cores, data-parallel over the 16384 pred edges, 2048/core):
  - No on-device gathers. pred_ei is host-known, so the host materializes the
    s-side and d-side adjacency rows in EDGE ORDER, 1-bit packed and
    pre-transposed to the SBUF layout: partition p's u16 word for superchunk C
    holds neighbors j = 2048*C + 16*p + 8*i + b at bit (8*i + b). Rows are
    streamed with plain HWDGE dma_start on one priority-ordered queue.
  - Device AND: one tensor_tensor(bitwise_and) per supertile computes s&d
    over the packed bits ([128, 6 words, E] u16).
  - Bit->fp8 expansion: for each bit-pair b (8 planes/supertile), ONE fused
    tensor_scalar((and <<|>> sh_b) & 0x0808) produces the fp8 e4m3 plane
    directly (byte 0x08 = 2^-6); bits (b, 8+b) land at byte positions (0, 1)
    = the DoubleRow 'two' sub-rows. Runs in the DVE 4x perf mode.
  - cn matmuls: fp8 DoubleRow, stationary [128, 2, 128] holding
    [count | hi limb of 4*z (64) | lo limb dims 0..62]; hi+lo summation
    happens inside the MLP W1 matmul via duplicated W1 rows. Stationaries
    are laid out b-major and DMA'd in per-b chunks so the first matmuls'
    weights arrive first.
  - Normalize: counts sit in PSUM row 0; the scalar engine copies them out
    with a +eps bias folded in (clamp-free: count-0 edges have exactly-zero
    psum rows, so 0 * (1/eps) = 0), DVE reciprocal_approx_fast inverts,
    gpsimd broadcasts 1/cnt to 128 partitions (its only op type — a second
    gpsimd op kind would trigger ~6us Q7 library reloads), DVE multiplies
    in the bf16 2x mode.
  - 4 supertiles of 512 edges; [128, 512] f32 PSUM tiles exactly fill a
    bank each (half-bank packing measurably slows PSUM accumulation).
"""

import numpy as np
import ml_dtypes
from contextlib import ExitStack

import concourse.bass as bass
import concourse.bacc as bacc
import concourse.mybir as mybir
import concourse.tile as tile

N_D, N_P = 8192, 4096
D_DIM, HID = 64, 128
E_PRED = 16384
N_CORES = 8
E_LOC = E_PRED // N_CORES          # 2048 edges per core
STS = (512, 512, 512, 512)         # supertile sizes
OFFS = tuple(int(x) for x in np.cumsum((0,) + STS))  # edge offsets
N_ET = len(STS)
C_A = N_D // 2048                  # 4 ddi superchunks (2048 entries each)
C_P = N_P // 2048                  # 2 dp superchunks
W_TOT = C_A + C_P                  # 6 u16 words per partition per row
G_A = C_A * 8                      # 32 ddi stationary groups
G_P = C_P * 8                      # 16 dp stationary groups

S_OUT = 2.0 ** -4                  # plane code (2^-6) * z premult (4) product
Z_SCALE = 4.0

FP8 = ml_dtypes.float8_e4m3
BF16 = ml_dtypes.bfloat16


def _pack_stationary(z: np.ndarray):
    """z [K, 64] f32 -> [128, (K/2048)*8*2*128] uint8 fp8 DoubleRow lhsT.
    B-MAJOR group order: g = b*(K/2048) + C holds rows k = 2048C + 16p + 8i + b
    at (partition p, sub-row i), scaled by Z_SCALE. Cols: 0 = count (Z_SCALE),
    [1:65] hi limb, [65:128] lo limb of dims 0..62."""
    K = z.shape[0]
    nC = K // 2048
    p = np.arange(128)[:, None]
    i = np.arange(2)[None, :]
    out = np.empty((8 * nC, 128, 2, 128), dtype=np.uint8)
    code = np.float32(Z_SCALE).astype(FP8).view(np.uint8)
    for C in range(nC):
        for b in range(8):
            ks = 2048 * C + 16 * p + 8 * i + b            # [128, 2]
            zsc = z[ks].astype(np.float32) * Z_SCALE      # [128, 2, 64]
            hi8 = zsc.astype(FP8)
            lo8 = (zsc - hi8.astype(np.float32)).astype(FP8)
            blk = np.zeros((128, 2, 128), dtype=np.uint8)
            blk[..., 0] = code
            blk[..., 1:65] = hi8.view(np.uint8)
            blk[..., 65:128] = lo8.view(np.uint8)[..., :63]
            out[b * nC + C] = blk
    return np.ascontiguousarray(out.transpose(1, 0, 2, 3).reshape(128, -1))


def build_body(tc, t):
    """Emit the per-core program. t: dict name -> AP of DRAM tensors."""
    nc = tc.nc
    dt = mybir.dt
    with ExitStack() as ctx:
        const = ctx.enter_context(tc.tile_pool(name="const", bufs=1))
        rows = ctx.enter_context(tc.tile_pool(name="rows", bufs=3))
        andp = ctx.enter_context(tc.tile_pool(name="andp", bufs=2))
        plnp = ctx.enter_context(tc.tile_pool(name="plnp", bufs=10))
        tails = ctx.enter_context(tc.tile_pool(name="tails", bufs=2))
        ps_cn = ctx.enter_context(tc.tile_pool(name="ps_cn", bufs=2, space="PSUM"))
        ps_ml = ctx.enter_context(tc.tile_pool(name="ps_ml", bufs=2, space="PSUM"))

        # Everything on the single sync HWDGE queue, in need-order.
        zds = const.tile([128, G_A * 2 * 128], dt.uint8)
        zps = const.tile([128, G_P * 2 * 128], dt.uint8)
        w1t = const.tile([128, 4 * HID], dt.uint16)
        w2t = const.tile([128, 1], dt.uint16)
        b1t = const.tile([128, 1], dt.float32)
        b2t = const.tile([1, 1], dt.float32)
        zsrc = const.tile([128, E_LOC], dt.uint16)
        zdst = const.tile([128, E_LOC], dt.uint16)

        st_state = {}

        def load(et, eng=None):
            n = STS[et]
            rs = rows.tile([128, W_TOT * n], dt.uint16, tag="rs", name=f"rs{et}")
            rd = rows.tile([128, W_TOT * n], dt.uint16, tag="rd", name=f"rd{et}")
            c0, c1 = W_TOT * OFFS[et], W_TOT * OFFS[et + 1]
            (eng or nc.sync).dma_start(rs[:], t["RS"][:, c0:c1])
            (eng or nc.sync).dma_start(rd[:], t["RD"][:, c0:c1])
            st_state[et] = (rs, rd)

        def load_stationary_chunk(b):
            nc.sync.dma_start(
                zds[:, b * C_A * 256:(b + 1) * C_A * 256],
                t["ZDS"][:, b * C_A * 256:(b + 1) * C_A * 256],
            )
            nc.sync.dma_start(
                zps[:, b * C_P * 256:(b + 1) * C_P * 256],
                t["ZPS"][:, b * C_P * 256:(b + 1) * C_P * 256],
            )

        # Priority-ordered prologue loads: first supertile rows, then the
        # first stationary chunk, MLP weights, next rows, remaining chunks.
        # The first rows go out on the scalar HWDGE queue — the ACT
        # sequencer boots ~2.3us before the sync one.
        load(0, eng=nc.scalar)
        load_stationary_chunk(0)
        nc.sync.dma_start(w1t[:], t["W1"][:, :])
        nc.sync.dma_start(b1t[:], t["B1"][:, :])
        nc.sync.dma_start(w2t[:], t["W2"][:, :])
        nc.sync.dma_start(b2t[:], t["B2"][:, :])
        load(1)
        for b in range(1, 8):
            load_stationary_chunk(b)
        nc.sync.dma_start(zsrc[:], t["ZS"][:, :])
        nc.sync.dma_start(zdst[:], t["ZD"][:, :])



        zds8 = zds[:].bitcast(dt.float8e4).rearrange(
            "p (g two c) -> p g two c", g=G_A, two=2
        )
        zps8 = zps[:].bitcast(dt.float8e4).rearrange(
            "p (g two c) -> p g two c", g=G_P, two=2
        )

        def and_(et):
            n = STS[et]
            rs, rd = st_state[et]
            at = andp.tile([128, W_TOT * n], dt.uint16, tag="and", name=f"and{et}")
            nc.vector.tensor_tensor(
                at[:], rs[:], rd[:], mybir.AluOpType.bitwise_and
            )
            st_state[et] = (rs, rd, at)

        def planes_mm(et):
            n = STS[et]
            _, _, at = st_state[et]
            psa = ps_cn.tile([128, n], dt.float32, tag="psa", name=f"psa{et}")
            psb = ps_cn.tile([128, n], dt.float32, tag="psb", name=f"psb{et}")
            for b in range(8):
                pl = plnp.tile([128, W_TOT * n], dt.uint16, tag="pl",
                               name=f"pl{et}_{b}")
                if b < 3:
                    nc.vector.tensor_scalar(
                        pl[:], at[:], 3 - b, 0x0808,
                        mybir.AluOpType.logical_shift_left,
                        mybir.AluOpType.bitwise_and,
                    )
                elif b == 3:
                    nc.vector.tensor_scalar(
                        pl[:], at[:], 0x0808, None,
                        mybir.AluOpType.bitwise_and,
                    )
                else:
                    nc.vector.tensor_scalar(
                        pl[:], at[:], b - 3, 0x0808,
                        mybir.AluOpType.logical_shift_right,
                        mybir.AluOpType.bitwise_and,
                    )
                mv = pl[:].bitcast(dt.float8e4).rearrange(
                    "p (w e two) -> p w two e", w=W_TOT, two=2
                )
                for C in range(C_A):
                    nc.tensor.matmul(
                        psa[:],
                        zds8[:, b * C_A + C],
                        mv[:, C],
                        start=(b == 0 and C == 0),
                        stop=(b == 7 and C == C_A - 1),
                        perf_mode=mybir.MatmulPerfMode.DoubleRow,
                    )
                for C in range(C_P):
                    nc.tensor.matmul(
                        psb[:],
                        zps8[:, b * C_P + C],
                        mv[:, C_A + C],
                        start=(b == 0 and C == 0),
                        stop=(b == 7 and C == C_P - 1),
                        perf_mode=mybir.MatmulPerfMode.DoubleRow,
                    )
            st_state[et] = (psa, psb)

        def tail(et):
            """Normalize + MLP + output for supertile et. The scalar engine
            moves the PSUM row-0 counts out with a +eps bias folded into an
            identity activation, so no clamp is needed: rec = 1/(cnt + eps).
            Count-0 edges have exactly-zero psum rows, so 0 * (1/eps) = 0
            matches the reference's max(cnt, 1); for cnt = n*S_OUT the
            relative bias is 2^-12/n."""
            n = STS[et]
            eps = float(S_OUT * 2.0 ** -12)
            psa, psb = st_state.pop(et)
            cnt = tails.tile([1, 2 * n], dt.float32, tag="cnt", bufs=1)
            nc.scalar.activation(
                cnt[0:1, 0:n], psa[0:1, :],
                mybir.ActivationFunctionType.Copy, bias=eps,
            )
            nc.scalar.activation(
                cnt[0:1, n:2 * n], psb[0:1, :],
                mybir.ActivationFunctionType.Copy, bias=eps,
            )
            rec = tails.tile([1, 2 * n], dt.float32, tag="rec", bufs=1)
            nc.vector.reciprocal_approx_fast(rec[:], cnt[:])
            rec_bf = tails.tile([1, 2 * n], dt.bfloat16, tag="recb", bufs=1)
            nc.scalar.copy(rec_bf[:], rec[:])
            rhs = {}
            for rel, ps, c0 in (("a", psa, 0), ("b", psb, n)):
                bcs = tails.tile([128, n], dt.bfloat16, tag=f"bc{rel}", bufs=1)
                nc.gpsimd.partition_broadcast(bcs[:], rec_bf[0:1, c0:c0 + n])
                pss = tails.tile([128, n], dt.bfloat16, tag=f"ps{rel}", bufs=1)
                nc.scalar.copy(pss[:], ps[:])
                rh = tails.tile([128, n], dt.bfloat16, tag=f"rh{rel}")
                nc.vector.tensor_tensor(
                    rh[:], pss[:], bcs[:], mybir.AluOpType.mult
                )
                rhs[rel] = rh
            hps = ps_ml.tile([HID, n], dt.float32, tag="ps", bufs=1)
            rhs_chunks = (
                zsrc[:].bitcast(dt.bfloat16)[:, OFFS[et]:OFFS[et + 1]],
                zdst[:].bitcast(dt.bfloat16)[:, OFFS[et]:OFFS[et + 1]],
                rhs["a"][:],
                rhs["b"][:],
            )
            for j, r in enumerate(rhs_chunks):
                nc.tensor.matmul(
                    hps[:],
                    w1t[:].bitcast(dt.bfloat16)[:, HID * j:HID * (j + 1)],
                    r,
                    start=(j == 0),
                    stop=(j == 3),
                )
            hsb = tails.tile([HID, n], dt.bfloat16, tag="h", bufs=1)
            nc.scalar.activation(
                hsb[:], hps[:], mybir.ActivationFunctionType.Relu, bias=b1t[:, 0:1]
            )
            lps = ps_ml.tile([1, n], dt.float32, tag="lps", bufs=1)
            nc.tensor.matmul(
                lps[:], w2t[:].bitcast(dt.bfloat16), hsb[:], start=True, stop=True
            )
            osb = tails.tile([1, n], dt.float32, tag="osb", bufs=1)
            nc.scalar.activation(
                osb[:],
                lps[:],
                mybir.ActivationFunctionType.Sigmoid,
                bias=b2t[:, 0:1],
            )
            nc.sync.dma_start(t["OUT"][:, OFFS[et]:OFFS[et + 1]], osb[:])

        # Emission order per iteration keeps every engine queue stall-free:
        # DVE sees [AND(et), cnt+recip(et-1), extracts(et), mults(et-1)] —
        # the tail chain never sits in front of plane production, and the
        # PE sees [cn(et), MLP(et-1)] whose rhs is long ready.
        for et in range(N_ET):
            if et + 2 < N_ET:
                load(et + 2)
            and_(et)
            planes_mm(et)
            if et > 0:
                tail(et - 1)
        tail(N_ET - 1)


def build_program():
    nc = bacc.Bacc("TRN2", target_bir_lowering=False)
    dt = mybir.dt
    t = {
        "RS": nc.dram_tensor(
            "RS", [128, W_TOT * E_LOC], dt.uint16, kind="ExternalInput"
        ).ap(),
        "RD": nc.dram_tensor(
            "RD", [128, W_TOT * E_LOC], dt.uint16, kind="ExternalInput"
        ).ap(),
        "ZS": nc.dram_tensor("ZS", [128, E_LOC], dt.uint16, kind="ExternalInput").ap(),
        "ZD": nc.dram_tensor("ZD", [128, E_LOC], dt.uint16, kind="ExternalInput").ap(),
        "ZDS": nc.dram_tensor(
            "ZDS", [128, G_A * 2 * 128], dt.uint8, kind="ExternalInput"
        ).ap(),
        "ZPS": nc.dram_tensor(
            "ZPS", [128, G_P * 2 * 128], dt.uint8, kind="ExternalInput"
        ).ap(),
        "W1": nc.dram_tensor("W1", [128, 4 * HID], dt.uint16, kind="ExternalInput").ap(),
        "B1": nc.dram_tensor("B1", [HID, 1], dt.float32, kind="ExternalInput").ap(),
        "W2": nc.dram_tensor("W2", [HID, 1], dt.uint16, kind="ExternalInput").ap(),
        "B2": nc.dram_tensor("B2", [1, 1], dt.float32, kind="ExternalInput").ap(),
        "OUT": nc.dram_tensor("OUT", [1, E_LOC], dt.float32, kind="ExternalOutput").ap(),
    }
    with tile.TileContext(nc) as tc:
        build_body(tc, t)
    nc.compile()
    return nc


def host_prep(z_drug, z_protein, ddi_ei, dp_ei, pred_ei, W1, b1, W2, b2):
    """Build the 8 per-core input maps (all numpy, no device work)."""
    z_drug = np.asarray(z_drug, np.float32)
    z_protein = np.asarray(z_protein, np.float32)
    ddi_ei = np.asarray(ddi_ei, np.int64)
    dp_ei = np.asarray(dp_ei, np.int64)
    pred_ei = np.asarray(pred_ei, np.int64)

    A_ddi = np.zeros((N_D, N_D), dtype=np.uint8)
    A_ddi[ddi_ei[0], ddi_ei[1]] = 1
    A_ddi[ddi_ei[1], ddi_ei[0]] = 1
    A_dp = np.zeros((N_D, N_P), dtype=np.uint8)
    A_dp[dp_ei[0], dp_ei[1]] = 1

    # 1-bit packed, pre-transposed row content: word (row, w, p) holds
    # neighbor j = 2048*w + 16*p + beta at bit beta (little-endian).
    P_ddi = np.packbits(
        A_ddi.reshape(N_D, C_A, 128, 16), axis=-1, bitorder="little"
    ).view(np.uint16)[..., 0]
    P_dp = np.packbits(
        A_dp.reshape(N_D, C_P, 128, 16), axis=-1, bitorder="little"
    ).view(np.uint16)[..., 0]
    P_all = np.concatenate([P_ddi, P_dp], axis=1)       # [N_D, 6, 128]

    zb = np.zeros((128, N_D), dtype=np.uint16)
    zb[0:D_DIM] = z_drug.astype(BF16).view(np.uint16).T

    ZDS = _pack_stationary(z_drug)
    ZPS = _pack_stationary(z_protein)

    # W1 blocks [128, HID] bf16: block0/1 rows 0:64 = z dims; block2/3 row 0
    # = 0 (count row), rows 1:65 = hi-limb dims, 65:128 = lo-limb dims 0..62
    # (duplicated weights sum the limbs).
    W1f = np.asarray(W1, np.float32)  # [256, HID]
    blocks = np.zeros((4, 128, HID), dtype=np.float32)
    blocks[0, 0:64] = W1f[0:64]
    blocks[1, 0:64] = W1f[64:128]
    blocks[2, 1:65] = W1f[128:192]
    blocks[2, 65:128] = W1f[128:191]
    blocks[3, 1:65] = W1f[192:256]
    blocks[3, 65:128] = W1f[192:255]
    W1p = np.ascontiguousarray(
        blocks.astype(BF16).view(np.uint16).transpose(1, 0, 2).reshape(128, 4 * HID)
    )
    B1 = np.asarray(b1, np.float32).reshape(HID, 1)
    W2p = np.asarray(W2, np.float32).reshape(HID, 1).astype(BF16).view(np.uint16)
    B2 = np.asarray(b2, np.float32).reshape(1, 1)

    in_maps = []
    for core in range(N_CORES):
        s = pred_ei[0, core * E_LOC:(core + 1) * E_LOC]
        d = pred_ei[1, core * E_LOC:(core + 1) * E_LOC]
        # per-supertile [n, 6, 128] -> [128, w, e] blocks, concatenated
        RS = np.concatenate(
            [
                P_all[s[OFFS[et]:OFFS[et + 1]]].transpose(2, 1, 0).reshape(128, -1)
                for et in range(N_ET)
            ],
            axis=1,
        )
        RD = np.concatenate(
            [
                P_all[d[OFFS[et]:OFFS[et + 1]]].transpose(2, 1, 0).reshape(128, -1)
                for et in range(N_ET)
            ],
            axis=1,
        )
        in_maps.append(
            {
                "RS": np.ascontiguousarray(RS),
                "RD": np.ascontiguousarray(RD),
                "ZS": np.ascontiguousarray(zb[:, s]),
                "ZD": np.ascontiguousarray(zb[:, d]),
                "ZDS": ZDS,
                "ZPS": ZPS,
                "W1": W1p,
                "B1": B1,
                "W2": W2p,
                "B2": B2,
            }
        )
    return in_maps


def kernel(z_drug, z_protein, ddi_ei, dp_ei, pred_ei, W1, b1, W2, b2, _profile=None):
    from concourse.bass_utils import run_bass_kernel_spmd

    in_maps = host_prep(z_drug, z_protein, ddi_ei, dp_ei, pred_ei, W1, b1, W2, b2)
    nc = build_program()
    res = run_bass_kernel_spmd(
        nc,
        in_maps,
        core_ids=list(range(N_CORES)),
        **({} if _profile is None else _profile),
    )
    if _profile is not None:
        kernel.last_results = res
    out = np.concatenate([r["OUT"].reshape(-1) for r in res.results])
    return out.astype(np.float32)
